# revision 10
# baseline (speedup 1.0000x reference)
import sys

import numpy as np

sys.path.insert(0, "/opt/trn_rl_repo")

from concourse import bacc, bass, tile  # noqa: E402
from concourse.bass_utils import run_bass_kernel_spmd  # noqa: E402

mybir = bass.mybir
FP16 = mybir.dt.float16
FP32 = mybir.dt.float32

B, C, H, W = 4, 256, 192, 192
NUM_HEADS = 8
DQK = 16
BLOCK = 8
HALO = 2
WIN = 12
NW = W // BLOCK  # 24 block-cols
SR = 100  # strip rows incl 2+2 halo
NRB = 12  # block-rows per strip
N_CORES = 8


def _build_nc():
    nc = bacc.Bacc(trn_type="TRN2")
    x_d = nc.declare_dram_parameter("x", [128, 2, SR, W], FP16, isOutput=False)
    wq_d = [
        nc.declare_dram_parameter(f"wq{p}", [128, 2, 128], FP16, isOutput=False)
        for p in range(4)
    ]
    wk_d = nc.declare_dram_parameter("wk", [128, 2, 128], FP16, isOutput=False)
    # out[p, a, r, q, j, n] = scores for head 4a+p, strip-row r, block j
    out_d = nc.declare_dram_parameter(
        "out", [4, 2, NRB, 64, NW, 144], FP16, isOutput=True
    )

    copy_i = [0]

    def copy_op(out, in_):
        # scores copies are split DVE:ACT 2:1
        if copy_i[0] % 3 == 2:
            nc.scalar.copy(out=out, in_=in_)
        else:
            nc.vector.tensor_copy(out=out, in_=in_)
        copy_i[0] += 1

    dma_i = [0]

    def out_dma(out, in_):
        eng = nc.gpsimd if dma_i[0] % 2 else nc.sync
        eng.dma_start(out=out, in_=in_)
        dma_i[0] += 1

    with tile.TileContext(nc) as tc:
        with (
            tc.tile_pool(name="wpool", bufs=1) as wpool,
            tc.tile_pool(name="kpool", bufs=1) as kpool,
            tc.tile_pool(name="xpool", bufs=2) as xpool,
            tc.tile_pool(name="qpool", bufs=1) as qpool,
            tc.tile_pool(name="spool", bufs=2) as spool,
            tc.tile_pool(name="pq_pool", bufs=2, space="PSUM") as pq_pool,
            tc.tile_pool(name="pk_pool", bufs=2, space="PSUM") as pk_pool,
            tc.tile_pool(name="ps_pool", bufs=4, space="PSUM") as ps_pool,
        ):
            wq_sb = [
                wpool.tile([128, 2, 128], FP16, name=f"wq{p}", tag=f"wq{p}")
                for p in range(4)
            ]
            wk_sb = wpool.tile([128, 2, 128], FP16, name="wk", tag="wk")
            for p in range(4):
                nc.sync.dma_start(out=wq_sb[p][:, :, :], in_=wq_d[p][:, :, :])
            nc.sync.dma_start(out=wk_sb[:, :, :], in_=wk_d[:, :, :])

            k_buf = kpool.tile([128, SR, W + 4], FP16, name="k_buf", tag="k_buf")
            nc.vector.memset(k_buf[:, :, 0:2], 0.0)
            nc.vector.memset(k_buf[:, :, W + 2 : W + 4], 0.0)

            # block-diagonal q layout: cols 0:64 head p (chans 0:64),
            # cols 64:128 head 4+p (chans 64:128); off-diag quadrants zero
            q_pad = [
                qpool.tile([128, NW, 128], FP16, name=f"q_pad{p}", tag=f"qpad{p}")
                for p in range(4)
            ]
            for p in range(4):
                nc.vector.memset(q_pad[p][0:64, :, 64:128], 0.0)
                nc.vector.memset(q_pad[p][64:128, :, 0:64], 0.0)

            for r in range(NRB):
                x_t = xpool.tile([128, 2, WIN, W], FP16, name="x_t", tag="x")
                nc.sync.dma_start(
                    out=x_t[:, :, :, :], in_=x_d[:, :, 8 * r : 8 * r + WIN, :]
                )

                # ---- k-conv: write k_buf rows for this iteration ----
                if r == 0:
                    row_pairs = [0, 2, 4, 6, 8, 10]
                else:
                    row_pairs = [8 * r + 4, 8 * r + 6, 8 * r + 8, 8 * r + 10]
                for row0 in row_pairs:
                    t_off = row0 - 8 * r
                    pk = pk_pool.tile([128, 2, W], FP32, name="pk")
                    for kc in range(2):
                        nc.tensor.matmul(
                            pk[:, :, :],
                            wk_sb[:, kc, :],
                            x_t[:, kc, t_off : t_off + 2, :],
                            start=(kc == 0),
                            stop=(kc == 1),
                        )
                    nc.vector.tensor_copy(
                        out=k_buf[:, row0 : row0 + 2, 2 : W + 2], in_=pk[:, :, :]
                    )

                # ---- q-conv: wq_sb[p] emits head p at chans 16p..16p+16 and
                # head 4+p at 64+16p..64+16p+16, zeros elsewhere ----
                for g3 in range(3):
                    rhs = [
                        x_t[:, kc, 2:10, :].rearrange("p r (j c) -> p j r c", c=8)[
                            :, 8 * g3 : 8 * g3 + 8
                        ]
                        for kc in range(2)
                    ]
                    for p in range(4):
                        pq = pq_pool.tile([128, 8, 64], FP32, name="pq")
                        for kc in range(2):
                            nc.tensor.matmul(
                                pq[:, :, :],
                                wq_sb[p][:, kc, :],
                                rhs[kc],
                                start=(kc == 0),
                                stop=(kc == 1),
                            )
                        nc.vector.tensor_copy(
                            out=q_pad[p][0:64, 8 * g3 : 8 * g3 + 8, 0:64],
                            in_=pq[0:64, :, :],
                        )
                        nc.vector.tensor_copy(
                            out=q_pad[p][64:128, 8 * g3 : 8 * g3 + 8, 64:128],
                            in_=pq[64:128, :, :],
                        )

                # ---- attention: one K=128/M=128 matmul per (head-pair, block)
                st = spool.tile([128, 4, NW, 144], FP16, name="st", tag="st")
                for p in range(4):
                    for jg in range(8):
                        ps = ps_pool.tile([128, 3, 144], FP32, name="ps")
                        for i in range(3):
                            j = 3 * jg + i
                            nc.tensor.matmul(
                                ps[:, i, :],
                                q_pad[p][:, j, :],
                                k_buf[:, 8 * r : 8 * r + WIN, 8 * j : 8 * j + WIN],
                                start=True,
                                stop=True,
                            )
                        copy_op(
                            out=st[:, p, 3 * jg : 3 * jg + 3, :], in_=ps[:, :, :]
                        )

                # ---- DMA scores out ----
                for p in range(4):
                    for a in range(2):
                        out_dma(
                            out=out_d[p, a, r, :, :, :],
                            in_=st[64 * a : 64 * a + 64, p, :, :],
                        )

    nc.finalize()
    return nc


def _host_inputs(x, q_w, kv_w):
    xf = np.asarray(x, dtype=np.float16)
    qwf = np.asarray(q_w, dtype=np.float16)
    kvf = np.asarray(kv_w, dtype=np.float16)

    # wq[p]: zero-padded per-head layout, m = 64*g + 16*pp + d,
    # nonzero only for pp == p, holding head 4*g + p
    qw_k = qwf.reshape(NUM_HEADS, DQK, 2, 128).transpose(2, 3, 0, 1)  # [kc,c,h,d]
    wq_h = []
    for p in range(4):
        wq = np.zeros((2, 128, 2, 4, 16), np.float16)  # [kc,c,g,pp,d]
        wq[:, :, :, p, :] = qw_k[:, :, p::4, :]
        wq_h.append(
            np.ascontiguousarray(wq.reshape(2, 128, 128).transpose(1, 0, 2))
        )

    # wk: k rows of kv_w, m = h*16 + d
    kk = kvf.reshape(NUM_HEADS, 48, 2, 128)[:, 0:DQK]  # [h,d,kc,c]
    wk_h = np.ascontiguousarray(kk.transpose(3, 2, 0, 1).reshape(128, 2, 128))

    in_maps = []
    for core in range(N_CORES):
        b, s = core // 2, core % 2
        strip = np.zeros((C, SR, W), np.float16)
        if s == 0:
            strip[:, 2:100] = xf[b, :, 0:98]
        else:
            strip[:, 0:98] = xf[b, :, 94:192]
        x_h = np.ascontiguousarray(
            strip.reshape(2, 128, SR, W).transpose(1, 0, 2, 3)
        )
        m = {"x": x_h, "wk": wk_h}
        for p in range(4):
            m[f"wq{p}"] = wq_h[p]
        in_maps.append(m)
    return in_maps


def _gather(results):
    full = np.empty((B, NUM_HEADS, 24, 24, 64, 144), np.float32)
    for core in range(N_CORES):
        b, s = core // 2, core % 2
        o = results[core]["out"]  # [p, a, r, q, j, n] fp16
        o = o.transpose(1, 0, 2, 4, 3, 5).reshape(NUM_HEADS, NRB, NW, 64, 144)
        full[b, :, 12 * s : 12 * s + 12] = o.astype(np.float32)
    return np.ascontiguousarray(full.reshape(B * NUM_HEADS, 576, 64, 144))


def run(x, q_w, kv_w, trace=False):
    nc = _build_nc()
    in_maps = _host_inputs(x, q_w, kv_w)
    res = run_bass_kernel_spmd(nc, in_maps, list(range(N_CORES)), trace=trace)
    return _gather(res.results), res


def kernel(**inputs):
    out, _ = run(inputs["x"], inputs["q_w"], inputs["kv_w"])
    return out


# revision 22
# speedup vs baseline: 1.1178x; 1.1178x over previous
import sys

import numpy as np

sys.path.insert(0, "/opt/trn_rl_repo")

from concourse import bacc, bass, tile  # noqa: E402
from concourse.bass_utils import run_bass_kernel_spmd  # noqa: E402

mybir = bass.mybir
FP16 = mybir.dt.float16
FP32 = mybir.dt.float32

B, C, H, W = 4, 256, 192, 192
NUM_HEADS = 8
DQK = 16
BLOCK = 8
HALO = 2
WIN = 12
NW = W // BLOCK  # 24 block-cols
SR = 100  # strip rows incl 2+2 halo
NRB = 12  # block-rows per strip
N_CORES = 8


def _build_nc():
    nc = bacc.Bacc(trn_type="TRN2")
    x_d = nc.declare_dram_parameter("x", [128, 2, SR, W], FP16, isOutput=False)
    wq_d = [
        nc.declare_dram_parameter(f"wq{p}", [128, 2, 128], FP16, isOutput=False)
        for p in range(4)
    ]
    wk_d = nc.declare_dram_parameter("wk", [128, 2, 128], FP16, isOutput=False)
    # out[p, a, r, q, j, n] = scores for head 4a+p, strip-row r, block j
    out_d = nc.declare_dram_parameter(
        "out", [4, 2, NRB, 64, NW, 144], FP16, isOutput=True
    )

    copy_i = [0]

    def copy_op(out, in_):
        # q/scores PSUM->SBUF casts split 50:50 DVE:ACT
        if copy_i[0] % 2:
            nc.scalar.copy(out=out, in_=in_)
        else:
            nc.vector.tensor_copy(out=out, in_=in_)
        copy_i[0] += 1

    dma_i = [0]

    def out_dma(out, in_):
        eng = nc.gpsimd if dma_i[0] % 2 else nc.sync
        eng.dma_start(out=out, in_=in_)
        dma_i[0] += 1

    with tile.TileContext(nc) as tc:
        with (
            tc.tile_pool(name="wpool", bufs=1) as wpool,
            tc.tile_pool(name="kpool", bufs=1) as kpool,
            tc.tile_pool(name="xpool", bufs=2) as xpool,
            tc.tile_pool(name="qpool", bufs=1) as qpool,
            tc.tile_pool(name="spool", bufs=2) as spool,
            tc.tile_pool(name="pq_pool", bufs=2, space="PSUM") as pq_pool,
            tc.tile_pool(name="pk_pool", bufs=2, space="PSUM") as pk_pool,
            tc.tile_pool(name="ps_pool", bufs=4, space="PSUM") as ps_pool,
        ):
            wq_sb = [
                wpool.tile([128, 2, 128], FP16, name=f"wq{p}", tag=f"wq{p}")
                for p in range(4)
            ]
            wk_sb = wpool.tile([128, 2, 128], FP16, name="wk", tag="wk")
            for p in range(4):
                nc.sync.dma_start(out=wq_sb[p][:, :, :], in_=wq_d[p][:, :, :])
            nc.sync.dma_start(out=wk_sb[:, :, :], in_=wk_d[:, :, :])

            k_buf = kpool.tile([128, SR, W + 4], FP16, name="k_buf", tag="k_buf")
            nc.vector.memset(k_buf[:, :, 0:2], 0.0)
            nc.vector.memset(k_buf[:, :, W + 2 : W + 4], 0.0)

            # block-diagonal q layout: cols 0:64 head p (chans 0:64),
            # cols 64:128 head 4+p (chans 64:128); off-diag quadrants zero
            q_pad = [
                qpool.tile([128, NW, 128], FP16, name=f"q_pad{p}", tag=f"qpad{p}")
                for p in range(4)
            ]
            for p in range(4):
                nc.vector.memset(q_pad[p][0:64, :, 64:128], 0.0)
                nc.vector.memset(q_pad[p][64:128, :, 0:64], 0.0)

            for r in range(NRB):
                x_t = xpool.tile([128, 2, WIN, W], FP16, name="x_t", tag="x")
                nc.sync.dma_start(
                    out=x_t[:, :, :, :], in_=x_d[:, :, 8 * r : 8 * r + WIN, :]
                )

                # ---- k-conv: write k_buf rows for this iteration ----
                if r == 0:
                    row_pairs = [0, 2, 4, 6, 8, 10]
                else:
                    row_pairs = [8 * r + 4, 8 * r + 6, 8 * r + 8, 8 * r + 10]
                for row0 in row_pairs:
                    t_off = row0 - 8 * r
                    pk = pk_pool.tile([128, 2, W], FP32, name="pk")
                    for kc in range(2):
                        nc.tensor.matmul(
                            pk[:, :, :],
                            wk_sb[:, kc, :],
                            x_t[:, kc, t_off : t_off + 2, :],
                            start=(kc == 0),
                            stop=(kc == 1),
                        )
                    nc.vector.tensor_copy(
                        out=k_buf[:, row0 : row0 + 2, 2 : W + 2], in_=pk[:, :, :]
                    )

                # ---- q-conv: wq_sb[p] emits head p at chans 16p..16p+16 and
                # head 4+p at 64+16p..64+16p+16, zeros elsewhere ----
                for g3 in range(3):
                    rhs = [
                        x_t[:, kc, 2:10, :].rearrange("p r (j c) -> p j r c", c=8)[
                            :, 8 * g3 : 8 * g3 + 8
                        ]
                        for kc in range(2)
                    ]
                    for p in range(4):
                        pq = pq_pool.tile([128, 8, 64], FP32, name="pq")
                        for kc in range(2):
                            nc.tensor.matmul(
                                pq[:, :, :],
                                wq_sb[p][:, kc, :],
                                rhs[kc],
                                start=(kc == 0),
                                stop=(kc == 1),
                            )
                        nc.vector.tensor_copy(
                            out=q_pad[p][0:64, 8 * g3 : 8 * g3 + 8, 0:64],
                            in_=pq[0:64, :, :],
                        )
                        nc.vector.tensor_copy(
                            out=q_pad[p][64:128, 8 * g3 : 8 * g3 + 8, 64:128],
                            in_=pq[64:128, :, :],
                        )

                # ---- attention: one K=128/M=128 matmul per (head-pair, block)
                st = spool.tile([128, 4, NW, 144], FP16, name="st", tag="st")
                for p in range(4):
                    for jg in range(8):
                        ps = ps_pool.tile([128, 3, 144], FP32, name="ps")
                        for i in range(3):
                            j = 3 * jg + i
                            nc.tensor.matmul(
                                ps[:, i, :],
                                q_pad[p][:, j, :],
                                k_buf[:, 8 * r : 8 * r + WIN, 8 * j : 8 * j + WIN],
                                start=True,
                                stop=True,
                            )
                        copy_op(
                            out=st[:, p, 3 * jg : 3 * jg + 3, :], in_=ps[:, :, :]
                        )

                # ---- DMA scores out ----
                for p in range(4):
                    for a in range(2):
                        out_dma(
                            out=out_d[p, a, r, :, :, :],
                            in_=st[64 * a : 64 * a + 64, p, :, :],
                        )

    nc.finalize()
    return nc


def _host_inputs(x, q_w, kv_w):
    xf = np.asarray(x, dtype=np.float16)
    qwf = np.asarray(q_w, dtype=np.float16)
    kvf = np.asarray(kv_w, dtype=np.float16)

    # wq[p]: zero-padded per-head layout, m = 64*g + 16*pp + d,
    # nonzero only for pp == p, holding head 4*g + p
    qw_k = qwf.reshape(NUM_HEADS, DQK, 2, 128).transpose(2, 3, 0, 1)  # [kc,c,h,d]
    wq_h = []
    for p in range(4):
        wq = np.zeros((2, 128, 2, 4, 16), np.float16)  # [kc,c,g,pp,d]
        wq[:, :, :, p, :] = qw_k[:, :, p::4, :]
        wq_h.append(
            np.ascontiguousarray(wq.reshape(2, 128, 128).transpose(1, 0, 2))
        )

    # wk: k rows of kv_w, m = h*16 + d
    kk = kvf.reshape(NUM_HEADS, 48, 2, 128)[:, 0:DQK]  # [h,d,kc,c]
    wk_h = np.ascontiguousarray(kk.transpose(3, 2, 0, 1).reshape(128, 2, 128))

    in_maps = []
    for core in range(N_CORES):
        b, s = core // 2, core % 2
        strip = np.zeros((C, SR, W), np.float16)
        if s == 0:
            strip[:, 2:100] = xf[b, :, 0:98]
        else:
            strip[:, 0:98] = xf[b, :, 94:192]
        x_h = np.ascontiguousarray(
            strip.reshape(2, 128, SR, W).transpose(1, 0, 2, 3)
        )
        m = {"x": x_h, "wk": wk_h}
        for p in range(4):
            m[f"wq{p}"] = wq_h[p]
        in_maps.append(m)
    return in_maps


def _gather(results):
    full = np.empty((B, NUM_HEADS, 24, 24, 64, 144), np.float32)
    for core in range(N_CORES):
        b, s = core // 2, core % 2
        o = results[core]["out"]  # [p, a, r, q, j, n] fp16
        o = o.transpose(1, 0, 2, 4, 3, 5).reshape(NUM_HEADS, NRB, NW, 64, 144)
        full[b, :, 12 * s : 12 * s + 12] = o.astype(np.float32)
    return np.ascontiguousarray(full.reshape(B * NUM_HEADS, 576, 64, 144))


def run(x, q_w, kv_w, trace=False):
    nc = _build_nc()
    in_maps = _host_inputs(x, q_w, kv_w)
    res = run_bass_kernel_spmd(nc, in_maps, list(range(N_CORES)), trace=trace)
    return _gather(res.results), res


def kernel(**inputs):
    out, _ = run(inputs["x"], inputs["q_w"], inputs["kv_w"])
    return out
